# revision 1
# baseline (speedup 1.0000x reference)
"""Causal self-attention (B=2, S=2048, D=1024, H=16) on 8 NeuronCores.

Sharding: data-parallel over batch (2 groups of 4 cores), tensor-parallel
over heads within a group (4 heads / core). Each core computes Q/K/V
projections for its 4 heads, causal attention, and a partial output
projection through its slice of Wo; the 4 partial [2048, 1024] outputs per
batch are summed on the host.

v2 notes (vs the fp32r baseline):
  - x and [Wq|Wk|Wv] ship as fp8e4 hi+lo residual pairs (host-prepared;
    W pre-scaled x32 so fp8 normals cover it). Projections run as 3-term
    DoubleRow fp8 matmuls (256-deep contraction at 0.5 cycles/col):
    X*W ~= Xh@Wh + Xl@Wh + Xh@Wl, rel err ~1e-3.
  - P = exp(scores) is written straight to bf16; PV and the out-projection
    run with bf16 operands (1 cycle/col, full rate).
  - Scores stay fp32r; diagonal key-tiles only compute columns >= dq
    (clamped to 256-wide so fp32r keeps full rate).
  - Softmax denominator rides the PV matmul as a fused ones-column
    (row 64 of each head's 65-row block); normalization multiplies read
    OTP/Rb straight out of PSUM.
  - y is converted to bf16 on the Pool engine and DMA'd out in
    [128, 2, 512] blocks; host upcasts and sums partials.
"""

import numpy as np
import ml_dtypes

import concourse.bass as bass
import concourse.mybir as mybir
import concourse.tile as tile
from concourse.bass_utils import run_bass_kernel_spmd

F32 = mybir.dt.float32
F32R = mybir.dt.float32r
BF16 = mybir.dt.bfloat16
F8 = mybir.dt.float8e4
AF = mybir.ActivationFunctionType
DR = mybir.MatmulPerfMode.DoubleRow

B, S, D, H = 2, 2048, 1024, 16
DH = D // H              # 64
HL = 4                   # heads per core
CL = HL * DH             # 256 channels per core
G = 4                    # cores per batch group
WSCALE = 32.0            # host pre-scale on Wq/Wk/Wv (fp8 range)
SCALE = (DH ** -0.5) / (WSCALE * WSCALE)   # folded into exp()
NQC = S // 512           # 4 q-chunks of 512
NKT = S // 128           # 16 key tiles of 128


def _split_excess_waits(nc, max_waits=1):
    """walrus in this toolchain rejects instructions carrying more than
    `max_waits` sem waits; split the excess onto preceding same-engine
    NoOps (sound: waits are monotone >= conditions hoisted earlier on
    the same engine)."""
    n_split = 0
    for f in nc.m.functions:
        for bb in f.blocks:
            out = []
            for inst in bb.instructions:
                si = inst.sync_info
                waits = list(si.on_wait) if si is not None and si.on_wait else []
                if len(waits) > max_waits:
                    head, keep = waits[:-max_waits], waits[-max_waits:]
                    for ci, start in enumerate(range(0, len(head), max_waits)):
                        nop = mybir.InstNoOp(
                            name=f"{inst.name}_wsplit{ci}",
                            sync_info=mybir.SyncInfo(
                                on_wait=head[start:start + max_waits],
                                on_update=[],
                            ),
                            engine=inst.engine,
                            bass_nofuse=True,
                        )
                        out.append(nop)
                        n_split += 1
                    si.on_wait = keep
                out.append(inst)
            if n_split:
                bb.instructions.clear()
                for i in out:
                    bb.instructions.append(i)
    return n_split


def _build_nc(split_waits=True):
    nc = bass.Bass()
    xh_d = nc.dram_tensor("xh", [D, S], F8, kind="ExternalInput")
    xl_d = nc.dram_tensor("xl", [D, S], F8, kind="ExternalInput")
    wh_d = nc.dram_tensor("wh", [D, 3 * CL], F8, kind="ExternalInput")
    wl_d = nc.dram_tensor("wl", [D, 3 * CL], F8, kind="ExternalInput")
    wo_d = nc.dram_tensor("wo", [128, 2, D], BF16, kind="ExternalInput")
    mask_d = nc.dram_tensor("mask", [128, 2, 128], BF16, kind="ExternalInput")
    y_d = nc.dram_tensor("y", [S // 256, 2, 128, 2, 512], BF16,
                         kind="ExternalOutput")

    xh_r = xh_d.rearrange("(a p) s -> p a s", p=128)
    xl_r = xl_d.rearrange("(a p) s -> p a s", p=128)

    with tile.TileContext(nc) as tc:
        with tc.tile_pool(name="persist", bufs=1) as pp:
            # ---- persistent SBUF tensors -------------------------------
            wh_sb = pp.tile([128, 8, 3 * CL], F8)
            wl_sb = pp.tile([128, 8, 3 * CL], F8)
            xh_sb = pp.tile([128, 8, S], F8)
            xl_sb = pp.tile([128, 8, S], F8)
            wo_sb = pp.tile([128, 2, D], BF16)    # pair-major k-tiles
            mask_sb = pp.tile([128, 2, 128], BF16)  # tri m[k,q]=k<=q, x2 heads
            ones_sb = pp.tile([128, 128], F32)
            qt_sb = [pp.tile([128, S], F32R, name=f"qt{p}", tag=f"qt{p}")
                     for p in range(2)]
            kt_sb = [pp.tile([128, S], F32R, name=f"kt{p}", tag=f"kt{p}")
                     for p in range(2)]
            # V' per key-tile: 4x[64 v-cols + 1 ones-col], bf16
            vp_sb = pp.tile([128, NKT, 4 * 65], BF16)

            nc.vector.memset(ones_sb[:], 1.0)
            # ones-columns of V': fill everything with 1.0; the V copies
            # below overwrite the 64 data columns of each head block.
            nc.gpsimd.memset(vp_sb[:], 1.0)

            # ---- input DMAs (S-chunked so compute starts early) --------
            nc.sync.dma_start(
                wh_sb[:], wh_d.rearrange("(a p) m -> p a m", p=128))
            nc.sync.dma_start(
                wl_sb[:], wl_d.rearrange("(a p) m -> p a m", p=128))
            for c in range(NQC):
                cslc = slice(c * 512, (c + 1) * 512)
                nc.sync.dma_start(xh_sb[:, :, cslc], xh_r[:, :, cslc])
                nc.sync.dma_start(xl_sb[:, :, cslc], xl_r[:, :, cslc])
            nc.sync.dma_start(wo_sb[:], wo_d[:, :, :])
            nc.sync.dma_start(mask_sb[:], mask_d[:, :, :])

            # ---- unified pipeline ------------------------------------
            # One PSUM pool: tag "fast" (2 bufs x 2 banks) cycles the
            # short-lived tiles (QK/V projection chains, score tiles, Rb,
            # out-proj accumulators); tag "acc" (2 bufs x 2 banks) holds the
            # long-lived PV accumulators. Projection chains for chunk c+1,
            # normalize and out-projection for chunk qc-1 are interleaved
            # into chunk qc's attention kt-loop as PE filler so the exp
            # latency on ACT is hidden.
            with (
                tc.tile_pool(name="ps", bufs=2, space="PSUM") as psp,
                tc.tile_pool(name="pt", bufs=6) as ptp,
                tc.tile_pool(name="nrm", bufs=2) as nrm,
                tc.tile_pool(name="osb", bufs=4) as osb,
                nc.allow_low_precision(reason="bf16/fp8 pipeline"),
            ):
                xsb = {"h": xh_sb, "l": xl_sb}
                wsb = {"h": wh_sb, "l": wl_sb}

                def dr_terms(lhs_of, rhs_of, ps):
                    """3-term DoubleRow accumulation into psum region ps."""
                    terms = [("h", "h"), ("l", "h"), ("h", "l")]
                    n = len(terms) * 4
                    i = 0
                    for tl, tr in terms:
                        for k2 in range(4):
                            nc.tensor.matmul(
                                ps, lhs_of(tl, k2), rhs_of(tr, k2),
                                start=(i == 0), stop=(i == n - 1),
                                perf_mode=DR)
                            i += 1

                def fast_tile(name):
                    return psp.tile([128, 2, 512], F32, name=name, tag="fast")

                def emit_qk_chain(c, p):
                    cslc = slice(c * 512, (c + 1) * 512)
                    pslc = slice(p * 128, (p + 1) * 128)
                    kslc = slice(CL + p * 128, CL + (p + 1) * 128)
                    ps = fast_tile("psqk")
                    dr_terms(
                        lambda t, k2: wsb[t][:, 2 * k2:2 * k2 + 2, pslc],
                        lambda t, k2: xsb[t][:, 2 * k2:2 * k2 + 2, cslc],
                        ps[:, 0, :])
                    dr_terms(
                        lambda t, k2: wsb[t][:, 2 * k2:2 * k2 + 2, kslc],
                        lambda t, k2: xsb[t][:, 2 * k2:2 * k2 + 2, cslc],
                        ps[:, 1, :])
                    nc.vector.tensor_copy(qt_sb[p][:, cslc], ps[:, 0, :])
                    nc.vector.tensor_copy(kt_sb[p][:, cslc], ps[:, 1, :])

                def emit_v_chain(st):
                    sslc = slice(st * 128, (st + 1) * 128)
                    vslc = slice(2 * CL, 3 * CL)
                    ps = fast_tile("psv")
                    dr_terms(
                        lambda t, k2: xsb[t][:, 2 * k2:2 * k2 + 2, sslc],
                        lambda t, k2: wsb[t][:, 2 * k2:2 * k2 + 2, vslc],
                        ps[:, 0, 0:256])
                    nc.vector.tensor_copy(
                        vp_sb[:, st, :]
                        .rearrange("p (h e) -> p h e", e=65)[:, :, 0:64],
                        ps[:, 0, 0:256].rearrange("p (h d) -> p h d", d=64))

                def proj_items(c):
                    its = [lambda p=p: emit_qk_chain(c, p) for p in range(2)]
                    its += [lambda st=st: emit_v_chain(st)
                            for st in range(4 * c, 4 * (c + 1))]
                    return its

                def emit_st(qc, p, kt):
                    """scores + exp + mask for one key tile -> bf16 P."""
                    qlo = qc * 512
                    dq = max(0, kt * 128 - qlo)
                    s0 = min(dq, 256)   # fp32r needs >=256 free
                    ST = fast_tile("ST")
                    for hi in range(2):
                        hslc = slice(hi * 64, (hi + 1) * 64)
                        nc.tensor.matmul(
                            ST[:, hi, s0:],
                            kt_sb[p][hslc, kt * 128:(kt + 1) * 128],
                            qt_sb[p][hslc, qc * 512 + s0:(qc + 1) * 512],
                            start=True, stop=True)
                    PT = ptp.tile([128, 2, 512], BF16, tag="pt")
                    nc.scalar.activation(PT[:, :, dq:], ST[:, :, dq:],
                                         AF.Exp, scale=SCALE)
                    if kt * 128 >= qlo:      # diagonal: mask keys > query
                        if dq > 0:
                            nc.gpsimd.memset(PT[:, :, 0:dq], 0.0)
                        nc.gpsimd.tensor_mul(
                            PT[:, :, dq:dq + 128],
                            PT[:, :, dq:dq + 128], mask_sb[:])
                    return PT

                def emit_pv(p, kt, ktmax, PT, OTP):
                    # P@V (transposed): OT[c, q] += [V|1].T @ PT
                    # row 64 of each head region = softmax denominator
                    for hi in range(2):
                        bc = (2 * p + hi) * 65
                        nc.tensor.matmul(
                            OTP[0:65, hi, :], vp_sb[:, kt, bc:bc + 65],
                            PT[:, hi, :], start=(kt == 0),
                            stop=(kt == ktmax - 1))

                state = {}

                def norm_items(qc, otps):
                    # normalize rows 0:64 of each head by denom row 64;
                    # Rb broadcasts the reciprocal across partitions.
                    def item_a(p):
                        OTP = otps[p]
                        Ri = nrm.tile([128, 2, 512], F32R, tag="ri")
                        nc.vector.reciprocal(Ri[64:65, :, :],
                                             OTP[64:65, :, :])
                        OC = nrm.tile([64, 2, 512], BF16, tag="oc")
                        nc.scalar.copy(OC[:, :, :], OTP[0:64, :, :])
                        state[(qc, p)] = (Ri, OC)

                    def item_b(p):
                        Ri, OC = state.pop((qc, p))
                        if p == 0:
                            # OS2[0:64,p,:] even head; [64:128,p,:] odd head
                            state[qc] = (
                                osb.tile([128, 2, 512], BF16, name="OS2",
                                         tag="os"),
                                osb.tile([64, 2, 512], BF16, name="OSm",
                                         tag="osm"))
                        OS2, OSm = state[qc]
                        Rb = fast_tile("Rb")
                        for hi in range(2):
                            nc.tensor.matmul(
                                Rb[:, hi, :],
                                ones_sb.bitcast(F32R)[64:65, :],
                                Ri[64:65, hi, :], start=True, stop=True)
                        nc.vector.tensor_mul(OS2[0:64, p, :], OC[:, 0, :],
                                             Rb[0:64, 0, :])
                        nc.vector.tensor_mul(OSm[:, p, :], OC[:, 1, :],
                                             Rb[0:64, 1, :])
                        if p == 1:
                            nc.sync.dma_start(OS2[64:128, :, :],
                                              OSm[:, :, :])
                    return [lambda: item_a(0), lambda: item_a(1),
                            lambda: item_b(0), lambda: item_b(1)]

                def outproj_items(qc):
                    def item(sp2):
                        OS2, _ = state[qc]
                        ysb = osb.tile([128, 2, 2, 512], BF16, name="ysb",
                                       tag="ys")
                        for s2 in range(2):
                            st4 = 2 * sp2 + s2
                            sslc = slice(st4 * 128, (st4 + 1) * 128)
                            yp = fast_tile("yp")
                            for nch in range(2):
                                for kp in range(2):
                                    nc.tensor.matmul(
                                        yp[:, nch, :], OS2[:, kp, sslc],
                                        wo_sb[:, kp,
                                              nch * 512:(nch + 1) * 512],
                                        start=(kp == 0), stop=(kp == 1))
                            nc.vector.tensor_copy(ysb[:, s2, :, :], yp[:])
                        nc.sync.dma_start(
                            y_d[2 * qc + sp2].rearrange("s p n c -> p s n c"),
                            ysb[:])
                        if sp2 == 1:
                            state.pop(qc)
                    return [lambda: item(0), lambda: item(1)]

                # ---- master loop --------------------------------------
                for it in proj_items(0):
                    it()
                filler = []
                for qc in range(NQC):
                    ktmax = 4 * (qc + 1)
                    if qc + 1 < NQC:
                        filler.extend(proj_items(qc + 1))
                    otps = [psp.tile([65, 2, 512], F32, name=f"OT{qc}{p}",
                                     tag="acc") for p in range(2)]
                    n_iters = 2 * ktmax
                    n_fill = len(filler)
                    fi = 0
                    it_idx = 0
                    for p in range(2):
                        # software-pipeline: scores/exp run 2 key tiles
                        # ahead of PV; filler lands between ST and PV so
                        # PE has work while ACT computes exp.
                        pts = {}
                        for kt in range(ktmax):
                            pts[kt] = emit_st(qc, p, kt)
                            while fi < n_fill and \
                                    fi * n_iters < (it_idx + 1) * n_fill:
                                filler[fi]()
                                fi += 1
                            if kt >= 2:
                                emit_pv(p, kt - 2, ktmax, pts.pop(kt - 2),
                                        otps[p])
                            it_idx += 1
                        for kt in range(max(0, ktmax - 2), ktmax):
                            emit_pv(p, kt, ktmax, pts.pop(kt), otps[p])
                    while fi < n_fill:
                        filler[fi]()
                        fi += 1
                    filler = norm_items(qc, otps) + outproj_items(qc)
                # tail: last chunk's normalize + out-projection
                for it in filler:
                    it()

    if split_waits:
        _split_excess_waits(nc, max_waits=1)
    return nc


_NC = None


def _fp8_split(a):
    hi = a.astype(ml_dtypes.float8_e4m3)
    lo = (a - hi.astype(np.float32)).astype(ml_dtypes.float8_e4m3)
    return hi, lo


def _core_in_map(inputs, core, _xs_cache={}):
    x = np.asarray(inputs["x"], dtype=np.float32)
    Wq, Wk, Wv, Wo = (np.asarray(inputs[k], dtype=np.float32)
                      for k in ("Wq", "Wk", "Wv", "Wo"))
    b, g = divmod(core, G)
    csl = slice(g * CL, (g + 1) * CL)
    key = id(inputs)
    if key not in _xs_cache:
        _xs_cache.clear()
        _xs_cache[key] = [_fp8_split(np.ascontiguousarray(x[bb].T))
                          for bb in range(B)]
    xs = _xs_cache[key]
    w = np.concatenate(
        [Wq[csl, :].T, Wk[csl, :].T, Wv[csl, :].T], axis=1) * WSCALE
    whi, wlo = _fp8_split(np.ascontiguousarray(w))
    wo = np.ascontiguousarray(
        Wo[:, csl].T.reshape(2, 128, D).transpose(1, 0, 2)) / WSCALE
    tri = np.triu(np.ones((128, 128), dtype=np.float32))  # m[k,q] = k<=q
    mask16 = np.ascontiguousarray(
        np.stack([tri, tri], axis=1)).astype(ml_dtypes.bfloat16)
    return {
        "xh": xs[b][0], "xl": xs[b][1],
        "wh": whi, "wl": wlo,
        "wo": wo.astype(ml_dtypes.bfloat16),
        "mask": mask16,
    }


def kernel(x, Wq, Wk, Wv, Wo):
    global _NC
    if _NC is None:
        _NC = _build_nc()
    inputs = {"x": x, "Wq": Wq, "Wk": Wk, "Wv": Wv, "Wo": Wo}
    in_maps = [_core_in_map(inputs, core) for core in range(8)]
    res = run_bass_kernel_spmd(_NC, in_maps, list(range(8)))
    y = np.empty((B, S, D), dtype=np.float32)
    for b in range(B):
        acc = np.zeros((S // 256, 2, 128, 2, 512), dtype=np.float32)
        for g in range(G):
            acc += res.results[4 * b + g]["y"].astype(np.float32)
        y[b] = acc.reshape(S, D)
    return y



# revision 6
# speedup vs baseline: 1.1654x; 1.1654x over previous
"""Causal self-attention (B=2, S=2048, D=1024, H=16) on 8 NeuronCores.

Sharding: data-parallel over batch (2 groups of 4 cores), tensor-parallel
over heads within a group (4 heads / core). Each core computes Q/K/V
projections for its 4 heads, causal attention, and a partial output
projection through its slice of Wo; the 4 partial [2048, 1024] outputs per
batch are summed on the host.

v3 notes (vs v2):
  - PV runs transposed: OT[q, c] = sum_k P[k, q] V'[k, c] with the P tile
    (natural [key, query] layout) as the 128-wide stationary and V' (64
    v-cols + ones-col) streaming 65 columns per key tile. This streams
    65 cols per 128 queries instead of 512 cols per 65-wide stationary --
    2.3x fewer PE column-cycles for PV, same numerics.
  - Softmax denominators land on the partition (query) axis, so
    normalization is one broadcast-divide on DVE; the Rb reciprocal-
    broadcast matmuls of v2 are gone.
  - Normalized O^T [q, c] is transposed back to [c, q] for the output
    projection with 4 PE transpose matmuls (128 cols each) per (qc, p)
    through an identity stationary, then one DVE copy PSUM->SBUF.
  - PT memsets are gone: PV^T never reads below-diagonal slices.
  - Startup DMAs are spread across the SP/Activation/DVE HWDGE queues so
    the first projection matmul starts ~4us earlier.
  - x and [Wq|Wk|Wv] still ship as fp8e4 hi+lo residual pairs (host-
    prepared; W pre-scaled x32) with 3-term DoubleRow projections;
    P = exp(scores) is written straight to bf16; scores stay fp32r with
    diagonal key-tiles only computing columns >= dq (clamped to 256-wide
    so fp32r keeps full rate).
"""

import numpy as np
import ml_dtypes

import concourse.bass as bass
import concourse.mybir as mybir
import concourse.tile as tile
from concourse.bass_utils import run_bass_kernel_spmd

F32 = mybir.dt.float32
F32R = mybir.dt.float32r
BF16 = mybir.dt.bfloat16
F8 = mybir.dt.float8e4
AF = mybir.ActivationFunctionType
DR = mybir.MatmulPerfMode.DoubleRow
DIV = mybir.AluOpType.divide

B, S, D, H = 2, 2048, 1024, 16
DH = D // H              # 64
HL = 4                   # heads per core
CL = HL * DH             # 256 channels per core
G = 4                    # cores per batch group
WSCALE = 32.0            # host pre-scale on Wq/Wk/Wv (fp8 range)
SCALE = (DH ** -0.5) / (WSCALE * WSCALE)   # folded into exp()
NQC = S // 512           # 4 q-chunks of 512
NKT = S // 128           # 16 key tiles of 128


def _split_excess_waits(nc, max_waits=1):
    """walrus in this toolchain rejects instructions carrying more than
    `max_waits` sem waits; split the excess onto preceding same-engine
    NoOps (sound: waits are monotone >= conditions hoisted earlier on
    the same engine)."""
    n_split = 0
    for f in nc.m.functions:
        for bb in f.blocks:
            out = []
            for inst in bb.instructions:
                si = inst.sync_info
                waits = list(si.on_wait) if si is not None and si.on_wait else []
                if len(waits) > max_waits:
                    head, keep = waits[:-max_waits], waits[-max_waits:]
                    for ci, start in enumerate(range(0, len(head), max_waits)):
                        nop = mybir.InstNoOp(
                            name=f"{inst.name}_wsplit{ci}",
                            sync_info=mybir.SyncInfo(
                                on_wait=head[start:start + max_waits],
                                on_update=[],
                            ),
                            engine=inst.engine,
                            bass_nofuse=True,
                        )
                        out.append(nop)
                        n_split += 1
                    si.on_wait = keep
                out.append(inst)
            if n_split:
                bb.instructions.clear()
                for i in out:
                    bb.instructions.append(i)
    return n_split


def _build_nc(split_waits=True):
    nc = bass.Bass()
    xh_d = nc.dram_tensor("xh", [D, S], F8, kind="ExternalInput")
    xl_d = nc.dram_tensor("xl", [D, S], F8, kind="ExternalInput")
    wh_d = nc.dram_tensor("wh", [D, 3 * CL], F8, kind="ExternalInput")
    wl_d = nc.dram_tensor("wl", [D, 3 * CL], F8, kind="ExternalInput")
    wo_d = nc.dram_tensor("wo", [128, 2, D], BF16, kind="ExternalInput")
    mask_d = nc.dram_tensor("mask", [128, 2, 128], BF16, kind="ExternalInput")
    ident_d = nc.dram_tensor("ident", [128, 128], BF16, kind="ExternalInput")
    y_d = nc.dram_tensor("y", [S // 256, 2, 128, 2, 512], BF16,
                         kind="ExternalOutput")

    xh_r = xh_d.rearrange("(a p) s -> p a s", p=128)
    xl_r = xl_d.rearrange("(a p) s -> p a s", p=128)

    with tile.TileContext(nc) as tc:
        with tc.tile_pool(name="persist", bufs=1) as pp:
            # ---- persistent SBUF tensors -------------------------------
            wh_sb = pp.tile([128, 8, 3 * CL], F8)
            wl_sb = pp.tile([128, 8, 3 * CL], F8)
            xh_sb = pp.tile([128, 8, S], F8)
            xl_sb = pp.tile([128, 8, S], F8)
            wo_sb = pp.tile([128, 2, D], BF16)    # pair-major k-tiles
            mask_sb = pp.tile([128, 2, 128], BF16)  # tri m[k,q]=k<=q, x2 heads
            ident_sb = pp.tile([128, 128], BF16)
            qt_sb = [pp.tile([128, S], F32R, name=f"qt{p}", tag=f"qt{p}")
                     for p in range(2)]
            kt_sb = [pp.tile([128, S], F32R, name=f"kt{p}", tag=f"kt{p}")
                     for p in range(2)]
            # V' per key-tile: 4x[64 v-cols + 1 ones-col], bf16
            vp_sb = pp.tile([128, NKT, 4 * 65], BF16)

            # ones-columns of V': fill everything with 1.0; the V copies
            # below overwrite the 64 data columns of each head block.
            nc.gpsimd.memset(vp_sb[:], 1.0)

            # ---- input DMAs, spread across the SP and ACT HWDGE queues
            # so the first-chunk operands land early; W split QK|V so the
            # first projection chain isn't gated on the V columns ---------
            wh_r = wh_d.rearrange("(a p) m -> p a m", p=128)
            wl_r = wl_d.rearrange("(a p) m -> p a m", p=128)
            nc.sync.dma_start(xh_sb[:, :, 0:512], xh_r[:, :, 0:512])
            nc.scalar.dma_start(xl_sb[:, :, 0:512], xl_r[:, :, 0:512])
            nc.sync.dma_start(wh_sb[:, :, 0:512], wh_r[:, :, 0:512])
            nc.scalar.dma_start(wl_sb[:, :, 0:512], wl_r[:, :, 0:512])
            nc.sync.dma_start(wh_sb[:, :, 512:768], wh_r[:, :, 512:768])
            nc.scalar.dma_start(wl_sb[:, :, 512:768], wl_r[:, :, 512:768])
            nc.scalar.dma_start(mask_sb[:], mask_d[:, :, :])
            for c in range(1, NQC):
                cslc = slice(c * 512, (c + 1) * 512)
                nc.sync.dma_start(xh_sb[:, :, cslc], xh_r[:, :, cslc])
                nc.scalar.dma_start(xl_sb[:, :, cslc], xl_r[:, :, cslc])
            nc.scalar.dma_start(ident_sb[:], ident_d[:, :])
            nc.sync.dma_start(wo_sb[:], wo_d[:, :, :])

            # ---- unified pipeline ------------------------------------
            # One PSUM pool: tag "fast" (2 bufs x 2 banks) cycles the
            # short-lived tiles (QK/V projection chains, score tiles,
            # transpose outputs, out-proj accumulators); tag "acc"
            # (2 bufs x 2 banks) holds the PV^T accumulators. Projection
            # chains for chunk c+1, normalize/transpose and out-projection
            # for chunk qc-1 are interleaved into chunk qc's attention
            # kt-loop as PE filler so the exp latency on ACT is hidden.
            with (
                tc.tile_pool(name="ps", bufs=2, space="PSUM") as psp,
                tc.tile_pool(name="pt", bufs=6) as ptp,
                tc.tile_pool(name="nrm", bufs=2) as nrm,
                tc.tile_pool(name="osb", bufs=4) as osb,
                nc.allow_low_precision(reason="bf16/fp8 pipeline"),
            ):
                xsb = {"h": xh_sb, "l": xl_sb}
                wsb = {"h": wh_sb, "l": wl_sb}

                def dr_terms(lhs_of, rhs_of, ps):
                    """3-term DoubleRow accumulation into psum region ps."""
                    terms = [("h", "h"), ("l", "h"), ("h", "l")]
                    n = len(terms) * 4
                    i = 0
                    for tl, tr in terms:
                        for k2 in range(4):
                            nc.tensor.matmul(
                                ps, lhs_of(tl, k2), rhs_of(tr, k2),
                                start=(i == 0), stop=(i == n - 1),
                                perf_mode=DR)
                            i += 1

                def fast_tile(name):
                    return psp.tile([128, 2, 512], F32, name=name, tag="fast")

                def emit_qk_chain(c, p):
                    cslc = slice(c * 512, (c + 1) * 512)
                    pslc = slice(p * 128, (p + 1) * 128)
                    kslc = slice(CL + p * 128, CL + (p + 1) * 128)
                    ps = fast_tile("psqk")
                    dr_terms(
                        lambda t, k2: wsb[t][:, 2 * k2:2 * k2 + 2, pslc],
                        lambda t, k2: xsb[t][:, 2 * k2:2 * k2 + 2, cslc],
                        ps[:, 0, :])
                    dr_terms(
                        lambda t, k2: wsb[t][:, 2 * k2:2 * k2 + 2, kslc],
                        lambda t, k2: xsb[t][:, 2 * k2:2 * k2 + 2, cslc],
                        ps[:, 1, :])
                    nc.vector.tensor_copy(qt_sb[p][:, cslc], ps[:, 0, :])
                    nc.vector.tensor_copy(kt_sb[p][:, cslc], ps[:, 1, :])

                def emit_v_chain(st):
                    sslc = slice(st * 128, (st + 1) * 128)
                    vslc = slice(2 * CL, 3 * CL)
                    ps = fast_tile("psv")
                    dr_terms(
                        lambda t, k2: xsb[t][:, 2 * k2:2 * k2 + 2, sslc],
                        lambda t, k2: wsb[t][:, 2 * k2:2 * k2 + 2, vslc],
                        ps[:, 0, 0:256])
                    nc.vector.tensor_copy(
                        vp_sb[:, st, :]
                        .rearrange("p (h e) -> p h e", e=65)[:, :, 0:64],
                        ps[:, 0, 0:256].rearrange("p (h d) -> p h d", d=64))

                def proj_items(c):
                    its = [lambda p=p: emit_qk_chain(c, p) for p in range(2)]
                    its += [lambda st=st: emit_v_chain(st)
                            for st in range(4 * c, 4 * (c + 1))]
                    return its

                def emit_st(qc, p, kt):
                    """scores + exp + mask for one key tile -> bf16 P."""
                    qlo = qc * 512
                    dq = max(0, kt * 128 - qlo)
                    s0 = min(dq, 256)   # fp32r needs >=256 free
                    ST = fast_tile("ST")
                    for hi in range(2):
                        hslc = slice(hi * 64, (hi + 1) * 64)
                        nc.tensor.matmul(
                            ST[:, hi, s0:],
                            kt_sb[p][hslc, kt * 128:(kt + 1) * 128],
                            qt_sb[p][hslc, qc * 512 + s0:(qc + 1) * 512],
                            start=True, stop=True)
                    PT = ptp.tile([128, 2, 512], BF16, tag="pt")
                    nc.scalar.activation(PT[:, :, dq:], ST[:, :, dq:],
                                         AF.Exp, scale=SCALE)
                    if dq > 0:      # diagonal block: mask keys > query
                        nc.gpsimd.tensor_mul(
                            PT[:, :, dq:dq + 128],
                            PT[:, :, dq:dq + 128], mask_sb[:])
                    elif kt * 128 == qlo:
                        nc.gpsimd.tensor_mul(
                            PT[:, :, 0:128],
                            PT[:, :, 0:128], mask_sb[:])
                    return PT

                def emit_pvT(qc, p, kt, PT, OTP):
                    # transposed PV: OT[q, c] += P[k, q].T @ [V|1]
                    # P tile is the 128-wide stationary; V' streams 65 cols.
                    # col 64 of each head block = softmax denominator.
                    # OTP is [128, 4, 2, 128] f32 = exactly 2 psum banks with
                    # each (j, hi) slice 512B-aligned; one accumulation group
                    # per bank: start on the bank's first matmul (lazy-zeroes
                    # the whole bank), stop on its last (diagonal of the
                    # bank's last q-tile, hi=1).
                    for j in range(4):
                        qt = 4 * qc + j
                        if qt < kt:
                            continue
                        for hi in range(2):
                            bc = (2 * p + hi) * 65
                            nc.tensor.matmul(
                                OTP[:, j, hi, 0:65],
                                PT[:, hi, j * 128:(j + 1) * 128],
                                vp_sb[:, kt, bc:bc + 65],
                                start=(kt == 0 and hi == 0 and j % 2 == 0),
                                stop=(kt == qt and hi == 1 and j % 2 == 1))

                state = {}

                def norm_items(qc, otps):
                    # O^T[q, c] = OT[q, c] * (1/denom[q]) -- denominators sit
                    # on the partition (query) axis, so this is the HW-native
                    # per-partition tensor_scalar scale; then transpose back
                    # to [c, q] on PE via the identity stationary and copy
                    # PSUM->SBUF.
                    def item_a(p):
                        OTP = otps[p]
                        rec = nrm.tile([128, 4, 2, 1], F32, name="rec",
                                       tag="rec")
                        nc.vector.reciprocal(rec[:], OTP[:, :, :, 64:65])
                        Ob = nrm.tile([128, 4, 2, 64], BF16, name="Ob",
                                      tag="ob")
                        for j in range(4):
                            for hi in range(2):
                                nc.vector.tensor_scalar_mul(
                                    Ob[:, j, hi, :], OTP[:, j, hi, 0:64],
                                    rec[:, j, hi, :])
                        state[(qc, p)] = Ob

                    def item_b(p):
                        Ob = state.pop((qc, p))
                        if p == 0:
                            state[qc] = osb.tile([128, 2, 512], BF16,
                                                 name="OS2", tag="os")
                        OS2 = state[qc]
                        Tp = psp.tile([128, 4, 128], BF16, name="Tp",
                                      tag="fast")
                        for j in range(4):
                            nc.tensor.transpose(
                                Tp[:, j, :],
                                Ob[:, j, :, :].rearrange("p a b -> p (a b)"),
                                ident_sb[:])
                        nc.vector.tensor_copy(
                            OS2[:, p, :],
                            Tp[:].rearrange("p a b -> p (a b)"))
                    return [lambda: item_a(0), lambda: item_a(1),
                            lambda: item_b(0), lambda: item_b(1)]

                def outproj_items(qc):
                    def item(sp2):
                        OS2 = state[qc]
                        ysb = osb.tile([128, 2, 2, 512], BF16, name="ysb",
                                       tag="ys")
                        for s2 in range(2):
                            st4 = 2 * sp2 + s2
                            sslc = slice(st4 * 128, (st4 + 1) * 128)
                            yp = fast_tile("yp")
                            for nch in range(2):
                                for kp in range(2):
                                    nc.tensor.matmul(
                                        yp[:, nch, :], OS2[:, kp, sslc],
                                        wo_sb[:, kp,
                                              nch * 512:(nch + 1) * 512],
                                        start=(kp == 0), stop=(kp == 1))
                            nc.vector.tensor_copy(ysb[:, s2, :, :], yp[:])
                        nc.sync.dma_start(
                            y_d[2 * qc + sp2].rearrange("s p n c -> p s n c"),
                            ysb[:])
                        if sp2 == 1:
                            state.pop(qc)
                    return [lambda: item(0), lambda: item(1)]

                # ---- master loop --------------------------------------
                for it in proj_items(0):
                    it()
                filler = []
                for qc in range(NQC):
                    ktmax = 4 * (qc + 1)
                    if qc + 1 < NQC:
                        filler.extend(proj_items(qc + 1))
                    otps = [psp.tile([128, 4, 2, 128], F32, name=f"OT{qc}{p}",
                                     tag="acc") for p in range(2)]
                    n_iters = 2 * ktmax
                    n_fill = len(filler)
                    fi = 0
                    it_idx = 0
                    for p in range(2):
                        # software-pipeline: scores/exp run 2 key tiles
                        # ahead of PV^T; filler lands between ST and PV so
                        # PE has work while ACT computes exp.
                        pts = {}
                        for kt in range(ktmax):
                            pts[kt] = emit_st(qc, p, kt)
                            while fi < n_fill and \
                                    fi * n_iters < (it_idx + 1) * n_fill:
                                filler[fi]()
                                fi += 1
                            if kt >= 2:
                                emit_pvT(qc, p, kt - 2, pts.pop(kt - 2),
                                         otps[p])
                            it_idx += 1
                        for kt in range(max(0, ktmax - 2), ktmax):
                            emit_pvT(qc, p, kt, pts.pop(kt), otps[p])
                    while fi < n_fill:
                        filler[fi]()
                        fi += 1
                    filler = norm_items(qc, otps) + outproj_items(qc)
                # tail: last chunk's normalize + out-projection
                for it in filler:
                    it()

    if split_waits:
        _split_excess_waits(nc, max_waits=1)
    return nc


_NC = None


def _fp8_split(a):
    hi = a.astype(ml_dtypes.float8_e4m3)
    lo = (a - hi.astype(np.float32)).astype(ml_dtypes.float8_e4m3)
    return hi, lo


def _core_in_map(inputs, core, _xs_cache={}):
    x = np.asarray(inputs["x"], dtype=np.float32)
    Wq, Wk, Wv, Wo = (np.asarray(inputs[k], dtype=np.float32)
                      for k in ("Wq", "Wk", "Wv", "Wo"))
    b, g = divmod(core, G)
    csl = slice(g * CL, (g + 1) * CL)
    key = id(inputs)
    if key not in _xs_cache:
        _xs_cache.clear()
        _xs_cache[key] = [_fp8_split(np.ascontiguousarray(x[bb].T))
                          for bb in range(B)]
    xs = _xs_cache[key]
    w = np.concatenate(
        [Wq[csl, :].T, Wk[csl, :].T, Wv[csl, :].T], axis=1) * WSCALE
    whi, wlo = _fp8_split(np.ascontiguousarray(w))
    wo = np.ascontiguousarray(
        Wo[:, csl].T.reshape(2, 128, D).transpose(1, 0, 2)) / WSCALE
    tri = np.triu(np.ones((128, 128), dtype=np.float32))  # m[k,q] = k<=q
    mask16 = np.ascontiguousarray(
        np.stack([tri, tri], axis=1)).astype(ml_dtypes.bfloat16)
    ident = np.eye(128, dtype=np.float32).astype(ml_dtypes.bfloat16)
    return {
        "xh": xs[b][0], "xl": xs[b][1],
        "wh": whi, "wl": wlo,
        "wo": wo.astype(ml_dtypes.bfloat16),
        "mask": mask16,
        "ident": ident,
    }


def kernel(x, Wq, Wk, Wv, Wo):
    global _NC
    if _NC is None:
        _NC = _build_nc()
    inputs = {"x": x, "Wq": Wq, "Wk": Wk, "Wv": Wv, "Wo": Wo}
    in_maps = [_core_in_map(inputs, core) for core in range(8)]
    res = run_bass_kernel_spmd(_NC, in_maps, list(range(8)))
    y = np.empty((B, S, D), dtype=np.float32)
    for b in range(B):
        acc = np.zeros((S // 256, 2, 128, 2, 512), dtype=np.float32)
        for g in range(G):
            acc += res.results[4 * b + g]["y"].astype(np.float32)
        y[b] = acc.reshape(S, D)
    return y
